# revision 13
# baseline (speedup 1.0000x reference)
"""Trainium2 Bass kernel for nn_Discriminator (NeuralSort + MLP discriminator).

Computes, for x [64, 1024]:
    P_hat = softmax_j((scaling[i]*x_j - Bsum_j) / TAU)   (per sample)
    xs    = P_hat @ x
    out   = leaky(leaky(xs@W1.T + b1)@W2.T + b2) @ W3.T + b3

Data parallel over 8 NeuronCores: 8 samples per core.

Key structure (logits tile layout is [j on partitions, i on free]):
  - Bsum_j (the only fp32-sensitive reduction) is computed in COLUMN form
    on ACT (fused Abs+accum) / DVE (sub + abs-add-reduce) / Pool (sub) and
    enters the softmax as the exact-fp32 per-partition BIAS of the Exp
    activation -- no bf16 splits, no row flattens.
  - logits argexp: PE K=7 bf16 (t_h,t_m,t_l,t_h,t_m,mh,ml) x
    (a_h,a_h,a_h,a_l,a_l,1,1); t 3-way split, a 2-way exact, m 2-way.
  - row max on every-8th row: PE K=3 (t_h,t_m,-B_h) x (a_q,a_q,1);
    slack < 70 << 88 so sharing the group max is a valid softmax shift.
  - num/den: per i-chunk out [128,3] PSUM with lhsT = E chunks, rhs =
    (s_h, s_l, 1) -- lands directly in column form, no DRAM round-trip.
  - MLP: fp32 weights as lhsT [128k,128o], rhs = activations [128,8];
    out free dim 8 makes fp32 matmul cheap; Lrelu with per-partition
    bias does bias+leaky in one ACT op. b3 is added on the host.
"""

import numpy as np

import concourse.bass as bass
import concourse.bacc as bacc
import concourse.tile as tile
from concourse import mybir
from concourse.bass_utils import run_bass_kernel_spmd

F32 = mybir.dt.float32
BF16 = mybir.dt.bfloat16
ALU = mybir.AluOpType
ACTF = mybir.ActivationFunctionType

B, D = 64, 1024
NCORES = 8
S = B // NCORES          # samples per core
T = D // 128             # 128-tiles per sample dim
TAU = 1.0
NEG_SLOPE = 0.01
MAXSTRIDE = 8            # exact row max every MAXSTRIDE rows (D/8 = 128 rows)


def bf_split(x, n):
    """Split x into n bf16 parts (sum of parts -> x with ~8n mantissa bits)."""
    import ml_dtypes
    parts = []
    r = np.asarray(x, np.float32)
    for _ in range(n):
        p = r.astype(ml_dtypes.bfloat16)
        parts.append(p)
        r = r - p.astype(np.float32)
    return parts


def build_nc(loop_n: int = 1):
    nc = bacc.Bacc("TRN2", target_bir_lowering=False, debug=False,
                   enable_asserts=False, num_devices=NCORES)

    xs8 = nc.dram_tensor("xs8", [S, D], F32, kind="ExternalInput")
    l8_i = nc.dram_tensor("l8", [S, 8, D], BF16, kind="ExternalInput")
    swg_i = nc.dram_tensor("swg", [S, 128, T], F32, kind="ExternalInput")
    sw3_i = nc.dram_tensor("sw3", [S, 128, 3 * T], BF16, kind="ExternalInput")
    r6_i = nc.dram_tensor("r6", [6, D], BF16, kind="ExternalInput")
    r3m_i = nc.dram_tensor("r3m", [3, D // MAXSTRIDE], BF16, kind="ExternalInput")
    w1f_i = nc.dram_tensor("w1f", [D, D], F32, kind="ExternalInput")
    w2f_i = nc.dram_tensor("w2f", [D, D], F32, kind="ExternalInput")
    w3t_i = nc.dram_tensor("w3t", [D, 2], F32, kind="ExternalInput")
    b1r_i = nc.dram_tensor("b1r", [1, D], F32, kind="ExternalInput")
    b2r_i = nc.dram_tensor("b2r", [1, D], F32, kind="ExternalInput")
    on1_i = nc.dram_tensor("on1", [1, S], F32, kind="ExternalInput")
    out_t = nc.dram_tensor("out", [S, 2], F32, kind="ExternalOutput")

    args = (xs8, l8_i, swg_i, sw3_i, r6_i, r3m_i,
            w1f_i, w2f_i, w3t_i, b1r_i, b2r_i, on1_i, out_t)
    with tile.TileContext(nc) as tc:
        _body(nc, tc, args, loop_n)
    nc.finalize()
    return nc


def _body(nc, tc, args, loop_n):
    (xs8, l8_i, swg_i, sw3_i, r6_i, r3m_i,
     w1f_i, w2f_i, w3t_i, b1r_i, b2r_i, on1_i, out_t) = args
    from contextlib import ExitStack
    ctx = ExitStack()
    with ctx:
        consts = ctx.enter_context(tc.tile_pool(name="consts", bufs=1))
        per_s = ctx.enter_context(tc.tile_pool(name="per_s", bufs=3))
        big = ctx.enter_context(tc.tile_pool(name="big", bufs=4))
        epool = ctx.enter_context(tc.tile_pool(name="epool", bufs=11))
        dram = ctx.enter_context(tc.tile_pool(name="dram", bufs=4, space="DRAM"))

        # ---- small constants resident in SBUF ----
        r3m = consts.tile([3, D // MAXSTRIDE], BF16)
        nc.scalar.dma_start(out=r3m, in_=r3m_i[:, :])
        w3sb = consts.tile([128, 2 * T], F32)
        for g in range(T):
            nc.scalar.dma_start(out=w3sb[:, 2 * g:2 * g + 2],
                                in_=w3t_i[128 * g:128 * (g + 1), :])
        b1r = consts.tile([1, D], F32, tag="b1r")
        nc.scalar.dma_start(out=b1r, in_=b1r_i[:, :])
        b2r = consts.tile([1, D], F32, tag="b2r")
        nc.scalar.dma_start(out=b2r, in_=b2r_i[:, :])
        on1 = consts.tile([1, S], F32, tag="on1")
        nc.scalar.dma_start(out=on1, in_=on1_i[:, :])

        # big fp32 weights (issued lazily inside the sample loop so the
        # per-sample input DMAs win the head of the DMA queues)
        w1f = consts.tile([128, T * D], F32, tag="w1f")
        w2f = consts.tile([128, T * D], F32, tag="w2f")

        def load_w_chunk(wt, hnd, g):
            nc.scalar.dma_start(out=wt[:, g * D:(g + 1) * D],
                                in_=hnd[128 * g:128 * (g + 1), :])

        # per-core accumulator: num_h/num_l/den blocks, col = 64*r + 8*g + b
        ndAll = consts.tile([128, S * T * 3], F32, tag="ndAll")

        def one_rep():
            with tc.tile_pool(name="pbig", bufs=2, space="PSUM") as pbig, \
                 tc.tile_pool(name="pnd", bufs=2, space="PSUM") as pnd:
                for b in range(S):
                    _sample(nc, tc, b, xs8, l8_i, swg_i, sw3_i, r6_i, r3m,
                            per_s, big, epool, dram, pbig, pnd, ndAll)
                    # interleave weight-chunk loads behind early samples
                    if b >= 1 and b <= 4:
                        for g in range(2 * b - 2, 2 * b):
                            load_w_chunk(w1f, w1f_i, g)
                            load_w_chunk(w2f, w2f_i, g)
            with tc.tile_pool(name="pmlp", bufs=4, space="PSUM") as pmlp:
                _mlp(nc, tc, big, pmlp, ndAll, w1f, w2f, w3sb, b1r, b2r,
                     on1, out_t)

        if loop_n == 1:
            one_rep()
        else:
            with tc.For_i(0, loop_n, 1):
                one_rep()


def _sample(nc, tc, b, xs8, l8_i, swg_i, sw3_i, r6_i, r3m,
            per_s, big, epool, dram, pbig, pnd, ndAll):
    # ---- per-sample loads (sync queue; never the ACT queue) ----
    l8 = per_s.tile([8, D], BF16, tag="l8")
    nc.sync.dma_start(out=l8[:, :], in_=l8_i[b, :, :])
    # per-sample rhs: a-split rows (const) + per-sample m-hat rows 6:8
    ra = per_s.tile([8, D], BF16, tag="ra")
    nc.sync.dma_start(out=ra[0:6, :], in_=r6_i[:, :])
    swg = per_s.tile([128, T], F32, tag="swg")
    nc.sync.dma_start(out=swg, in_=swg_i[b, :, :])
    sw3 = per_s.tile([128, 3 * T], BF16, tag="sw3")
    nc.sync.dma_start(out=sw3, in_=sw3_i[b, :, :])
    # s broadcast to 128 partitions
    sbc = big.tile([128, D], F32, tag="sbc")
    src = xs8[b:b + 1, :]
    nc.sync.dma_start(out=sbc, in_=bass.AP(
        tensor=src.tensor, offset=src.offset, ap=[[0, 128]] + src.ap[1:]))

    # ---- Bsum columns (fp32): split tiles ACT / DVE / Pool ----
    bpos = per_s.tile([128, T], F32, tag="bpos")
    for g in range(T):
        eng = ("act", "act", "dve", "pool", "dve", "pool", "dve", "pool")[g]
        if eng == "act":
            gs = big.tile([128, D], F32, tag="gscr")
            nc.scalar.activation(out=gs, in_=sbc, func=ACTF.Abs,
                                 bias=swg[:, g:g + 1], scale=-1.0,
                                 accum_out=bpos[:, g:g + 1])
        else:
            ds = big.tile([128, D], F32, tag="gscr")
            if eng == "pool":
                nc.gpsimd.tensor_scalar_sub(out=ds, in0=sbc,
                                            scalar1=swg[:, g:g + 1])
            else:
                nc.vector.tensor_scalar_sub(out=ds, in0=sbc,
                                            scalar1=swg[:, g:g + 1])
            nc.vector.tensor_reduce(out=bpos[:, g:g + 1], in_=ds,
                                    axis=mybir.AxisListType.X, op=ALU.add,
                                    apply_absolute_value=True)
    bneg = per_s.tile([128, T], F32, tag="bneg")
    nc.vector.tensor_scalar_mul(out=bneg, in0=bpos, scalar1=-1.0)
    # -B_h in bf16 for the max matmul, flattened into l10 row 2
    bh = per_s.tile([128, T], BF16, tag="bh")
    nc.vector.tensor_copy(out=bh, in_=bneg)
    scrB = dram.tile([128, T], BF16, tag="scrB")
    nc.sync.dma_start(out=scrB, in_=bh)
    sap = scrB[:, :]
    nc.sync.dma_start(out=l8[2:3, :], in_=bass.AP(
        tensor=sap.tensor, offset=sap.offset, ap=[[1, T], [T, 128]]))

    # ---- row max on every-8th row (K=3 bf16) ----
    pa = pbig.tile([128, D], F32, tag="pbig")
    for c in range(2):
        nc.tensor.matmul(pa[:, 512 * c:512 * (c + 1)],
                         r3m[:, :],
                         l8[0:3, 512 * c:512 * (c + 1)],
                         start=True, stop=True)
    mqn = per_s.tile([128, 1], F32, tag="mqn")
    nc.vector.tensor_reduce(out=mqn, in_=pa[:, :],
                            axis=mybir.AxisListType.X, op=ALU.max,
                            negate=True)
    mq2 = per_s.tile([128, 2], BF16, tag="mq2")
    nc.vector.tensor_copy(out=mq2[:, 0:1], in_=mqn)
    mres = per_s.tile([128, 1], F32, tag="mres")
    nc.vector.tensor_sub(out=mres, in0=mqn, in1=mq2[:, 0:1])
    nc.vector.tensor_copy(out=mq2[:, 1:2], in_=mres)
    # i is device-ordered as i' = (i%8)*128 + i//8, so the per-group m-hat
    # row is the [1,128] group vector tiled 8x -- outer-dim repeat, fastest
    # dim contiguous (HW DGE requirement).
    for r in range(2):
        scrM = dram.tile([1, 128], BF16, tag=f"scrM{r}")
        nc.sync.dma_start(out=scrM, in_=mq2[:, r:r + 1])
        sap = scrM[:, :]
        nc.sync.dma_start(out=ra[6 + r:7 + r, :], in_=bass.AP(
            tensor=sap.tensor, offset=sap.offset,
            ap=[[0, MAXSTRIDE], [1, 128]]))

    # ---- argexp (K=7 bf16) + exp(+bias=-Bsum) + num/den ----
    ndp = pnd.tile([128, 3 * T], F32, tag="pnd")
    ets = []
    for g in range(T):
        pa = pbig.tile([128, D], F32, tag="pbig")
        for c in range(2):
            nc.tensor.matmul(pa[:, 512 * c:512 * (c + 1)],
                             l8[:, 128 * g:128 * (g + 1)],
                             ra[:, 512 * c:512 * (c + 1)],
                             start=True, stop=True)
        et = epool.tile([128, D], BF16, tag="et")
        nc.scalar.activation(out=et, in_=pa, func=ACTF.Exp,
                             bias=bneg[:, g:g + 1])
        ets.append(et)
    # one PSUM accumulation group per i-chunk at a time (bank constraint)
    for gi in range(T):
        for g in range(T):
            nc.tensor.matmul(ndp[:, 3 * gi:3 * gi + 3],
                             ets[g][:, 128 * gi:128 * (gi + 1)],
                             sw3[:, 3 * g:3 * g + 3],
                             start=(g == 0), stop=(g == T - 1))
    # columns land at ndAll col 64*r + 8*gi + b (r-major blocks)
    for r in range(3):
        nc.vector.tensor_copy(
            out=ndAll[:, 64 * r + b:64 * r + b + 57:8],
            in_=ndp[:, r:r + 22:3])


def _mlp(nc, tc, big, pmlp, ndAll, w1f, w2f, w3sb, b1r, b2r, on1, out_t):
    # xs[:, 8*g + b] = (num_h + num_l) / den; ndAll col = 64*r + 8*g + b
    rden = big.tile([128, S * T], F32, tag="rden")
    nc.vector.reciprocal(out=rden, in_=ndAll[:, 128:192])
    nsum = big.tile([128, S * T], F32, tag="nsum")
    nc.vector.tensor_add(out=nsum, in0=ndAll[:, 0:64], in1=ndAll[:, 64:128])
    h = big.tile([128, S * T], F32, tag="xsT")
    nc.vector.tensor_mul(out=h, in0=rden, in1=nsum)

    for wf, br, htag in ((w1f, b1r, "h1"), (w2f, b2r, "h2")):
        hn = big.tile([128, S * T], F32, tag=htag)
        for o in range(T):
            hp = pmlp.tile([128, S], F32, tag="hp")
            for g in range(T):
                nc.tensor.matmul(hp,
                                 wf[:, g * D + 128 * o:g * D + 128 * (o + 1)],
                                 h[:, g * S:(g + 1) * S],
                                 start=(g == 0), stop=False)
            nc.tensor.matmul(hp, br[0:1, 128 * o:128 * (o + 1)], on1,
                             start=False, stop=True)
            r99 = big.tile([128, S], F32, tag="r99")
            nc.scalar.activation(out=r99, in_=hp, func=ACTF.Relu,
                                 scale=1.0 - NEG_SLOPE)
            nc.vector.scalar_tensor_tensor(out=hn[:, o * S:(o + 1) * S],
                                           in0=hp, scalar=NEG_SLOPE,
                                           in1=r99, op0=ALU.mult,
                                           op1=ALU.add)
        h = hn

    op = pmlp.tile([S, 2], F32, tag="op")
    for g in range(T):
        nc.tensor.matmul(op, h[:, g * S:(g + 1) * S], w3sb[:, 2 * g:2 * g + 2],
                         start=(g == 0), stop=(g == T - 1))
    osb = big.tile([S, 2], F32, tag="osb")
    nc.vector.tensor_copy(out=osb, in_=op)
    nc.sync.dma_start(out=out_t[:, :], in_=osb)


# ---------------------------------------------------------------------------
# host-side input prep + entry point
# ---------------------------------------------------------------------------

def make_in_maps(x, W1, b1, W2, b2, W3, b3):
    import ml_dtypes
    BF = ml_dtypes.bfloat16
    x = np.ascontiguousarray(x, dtype=np.float32)
    scaling = (D - 1 - 2 * np.arange(D)).astype(np.float32)
    # device i-order: i' = (i%8)*128 + i//8  ->  i = (i'%128)*8 + i'//128
    iperm = (np.arange(D) % 128) * MAXSTRIDE + np.arange(D) // 128
    a_h, a_l = bf_split(scaling[iperm], 2)
    zero = np.zeros(D, BF)
    r6 = np.stack([a_h, a_h, zero, a_h, a_l, a_l]).astype(BF)
    a_q = np.ascontiguousarray(scaling[::MAXSTRIDE]).astype(BF)
    r3m = np.stack([a_q, a_q, np.ones(D // MAXSTRIDE, BF)]).astype(BF)
    w1f = np.ascontiguousarray(np.asarray(W1, np.float32).T[iperm])
    w2f = np.ascontiguousarray(W2.T, np.float32)
    w3t = np.ascontiguousarray(W3.T, dtype=np.float32)
    b1r = np.asarray(b1, np.float32).reshape(1, D)
    b2r = np.asarray(b2, np.float32).reshape(1, D)
    on1 = np.ones((1, S), np.float32)

    in_maps = []
    for c in range(NCORES):
        xs = x[c * S:(c + 1) * S]                      # [S, D]
        t = xs / TAU
        t_h, t_m, t_l = bf_split(t, 3)
        l8 = np.zeros((S, 8, D), BF)
        l8[:, 0], l8[:, 1] = t_h, t_m     # shared by max (w/ -B_h) and argexp
        l8[:, 3] = t_l
        l8[:, 4], l8[:, 5] = t_h, t_m                  # argexp a_l partners
        l8[:, 6], l8[:, 7] = 1.0, 1.0                  # m-hat partners
        cols = xs.reshape(S, T, 128).transpose(0, 2, 1)  # [S, 128, T]
        swg = np.ascontiguousarray(cols / TAU).astype(np.float32)
        s_h, s_l = bf_split(cols, 2)
        sw3 = np.zeros((S, 128, 3 * T), BF)
        sw3[:, :, 0::3] = s_h
        sw3[:, :, 1::3] = s_l
        sw3[:, :, 2::3] = 1.0
        in_maps.append({
            "xs8": np.ascontiguousarray(xs / TAU),
            "l8": l8, "swg": swg, "sw3": sw3,
            "r6": r6, "r3m": r3m,
            "w1f": w1f, "w2f": w2f, "w3t": w3t,
            "b1r": b1r, "b2r": b2r, "on1": on1,
        })
    return in_maps


_NC_CACHE = {}


def get_nc(loop_n: int = 1):
    if loop_n not in _NC_CACHE:
        _NC_CACHE[loop_n] = build_nc(loop_n)
    return _NC_CACHE[loop_n]


def kernel(x, W1, b1, W2, b2, W3, b3):
    nc = get_nc()
    in_maps = make_in_maps(np.asarray(x), np.asarray(W1), np.asarray(b1),
                           np.asarray(W2), np.asarray(b2), np.asarray(W3),
                           np.asarray(b3))
    res = run_bass_kernel_spmd(nc, in_maps, core_ids=list(range(NCORES)))
    out = np.concatenate([res.results[c]["out"] for c in range(NCORES)],
                         axis=0)
    return out + np.asarray(b3, np.float32)[None, :]
